# revision 1
# baseline (speedup 1.0000x reference)
"""Bidirectional cross-attention (nn_BidirectionalCross) on 8 Trainium2 cores.

Strategy: shard the 16 (batch, head) score matrices across 8 cores (2 heads
per core, one batch index per 4-core group). Each core computes, for each of
its heads, the score matrix in BOTH orientations ([l,s] and [s,l]) with
row-tiled K=32 matmuls, applies exp (unnormalized softmax - scores are O(1)
so no max subtraction needed) and the bool mask, and stream-accumulates the
attention messages plus their normalizers (ones-column trick). Outputs are
per-core partial merge projections, summed on the host (the unshard step).

v2: single bf16 matmul for sim (no hi/lo split; rel-err budget 2e-2 allows
it), plain bf16 merge (2 matmuls instead of 6), and both heads' accumulation
chains issued back-to-back so they can overlap on disjoint PE column groups.
"""

import sys

sys.path.insert(0, "/opt/trn_rl_repo")

import numpy as np
import ml_dtypes

import concourse.bacc as bacc
import concourse.tile as tile
from concourse import mybir
from concourse.bass_utils import run_bass_kernel_spmd

BF16 = mybir.dt.bfloat16
F32 = mybir.dt.float32
AF = mybir.ActivationFunctionType
ALU = mybir.AluOpType
bf16 = ml_dtypes.bfloat16

# Problem geometry (fixed by the harness).
N, L, C, H, FC, D = 2, 3000, 256, 256, 8, 32
SCALE = float((C // FC) ** 0.5)
DA = D + 1  # message dims + ones column

NCORES = 8
MASK_DMA_GPSIMD = False
MASK_GPS = False
MACC_BUFS = 1
LAG = 2  # groups of sim/exp/mask in flight ahead of accumulation

def _mm_orients():
    return {"dve": (), "mm": (0, 1), "split": (1,)}[MASK_MODE]

MASK_MODE = "dve"  # "dve" | "mm" | "split"  # fold mask into sim psum via identity-weight matmul (PE) instead of DVE mult


def build_core(P=3072, FB=1024, reps=1, skip=()):
    """Build the per-core Bass program.

    P:  padded sequence length (both l and s), multiple of 512.
    FB: free-axis block size (psum residency of the m^T accumulator),
        multiple of 512; P % FB == 0.
    reps: if >1, wrap the whole compute body in a hardware loop (for timing).
    """
    LS = 512  # l-super rows (4 x 128 partition tiles) per sim group
    NFB = P // FB
    NLS = P // LS
    NSC = FB // 512  # 512-wide chunks per f-block
    NT = P // 128  # 128-row tiles along the partition axis

    nc = bacc.Bacc(
        "TRN2",
        target_bir_lowering=False,
        debug=False,
        enable_asserts=True,
        num_devices=1,
    )

    xa = nc.dram_tensor("xa", [C, P], BF16, kind="ExternalInput")  # x0[n].T
    xb = nc.dram_tensor("xb", [C, P], BF16, kind="ExternalInput")  # x1[n].T
    # masks pre-tiled on host: [NFB, NLS2, 128, 3, FB]
    NLS2 = P // 384
    ma = nc.dram_tensor("ma", [NFB, NLS2, 128, 3, FB], BF16, kind="ExternalInput")
    mb = nc.dram_tensor("mb", [NFB, NLS2, 128, 3, FB], BF16, kind="ExternalInput")
    wq = nc.dram_tensor("wq", [C, 192], BF16, kind="ExternalInput")
    bq = nc.dram_tensor("bq", [96, 2], F32, kind="ExternalInput")
    wv = nc.dram_tensor("wv", [C, 2 * DA], BF16, kind="ExternalInput")
    bvt = nc.dram_tensor("bvt", [128, 2 * DA], F32, kind="ExternalInput")
    wmh = nc.dram_tensor("wmh", [64, 256], BF16, kind="ExternalInput")
    ident = (
        nc.dram_tensor("ident", [128, 128], BF16, kind="ExternalInput")
        if _mm_orients()
        else None
    )
    o0 = nc.dram_tensor("o0", [256, P], F32, kind="ExternalOutput")  # m0^T partial
    o1 = nc.dram_tensor("o1", [256, P], F32, kind="ExternalOutput")  # m1^T partial

    with tile.TileContext(nc) as tc:
        with (
            tc.tile_pool(name="const", bufs=1) as cp,
            tc.tile_pool(name="work", bufs=1) as wp,
            tc.tile_pool(name="ps", bufs=1, space="PSUM") as pp,
        ):
            # ---- constant loads (weights first so projections start early) ----
            wq_sb = cp.tile([128, 384], BF16, name="wq_sb")  # chunk c at cols 192c
            nc.sync.dma_start(wq_sb[:, 0:192], wq[0:128, :])
            nc.sync.dma_start(wq_sb[:, 192:384], wq[128:256, :])
            bq_sb = cp.tile([128, 2], F32, name="bq_sb")
            nc.sync.dma_start(bq_sb[0:96, :], bq[:, :])
            xsb = {}  # xsb[side][cchunk] -> [128, P] f32
            for side, dram in (("a", xa), ("b", xb)):
                for c in range(2):
                    xsb[(side, c)] = cp.tile([128, P], BF16, name=f"x{side}{c}")
            for ch in range(P // 1024):
                for side, dram in (("a", xa), ("b", xb)):
                    for c in range(2):
                        nc.sync.dma_start(
                            xsb[(side, c)][:, 1024 * ch : 1024 * (ch + 1)],
                            dram[128 * c : 128 * (c + 1), 1024 * ch : 1024 * (ch + 1)],
                        )
            wv_sb = cp.tile([128, 4 * DA], BF16, name="wv_sb")
            nc.sync.dma_start(wv_sb[:, 0 : 2 * DA], wv[0:128, :])
            nc.sync.dma_start(wv_sb[:, 2 * DA : 4 * DA], wv[128:256, :])

            bvt_sb = cp.tile([128, 2 * DA], F32, name="bvt_sb")
            nc.sync.dma_start(bvt_sb[:], bvt[:, :])
            if _mm_orients():
                id_sb = cp.tile([128, 128], BF16, name="id_sb")
                nc.sync.dma_start(id_sb[:], ident[:, :])
            wm_sb = {}
            for p in (0, 1):
                t = cp.tile([32, 256], BF16, name=f"wm_sb{p}")
                nc.sync.dma_start(t[:], wmh[32 * p : 32 * (p + 1), :])
                wm_sb[p] = t

            def body():
                # per-iteration q/v tensors, double-buffered so the next
                # iteration's projections overlap this iteration's tail
                qrep = {}  # (side, head) -> [128, P] bf16, replicated on 3 strips
                for side in ("a", "b"):
                    for h in (0, 1):
                        qrep[(side, h)] = wp.tile(
                            [128, P], BF16, tag=f"q{side}{h}", bufs=2, name=f"q{side}{h}"
                        )
                vaug = {}  # side -> [128, NT, 2*DA] bf16 (t-major v + ones col)
                for side in ("a", "b"):
                    vaug[side] = wp.tile(
                        [128, NT, 2 * DA], BF16, tag=f"v{side}", bufs=2, name=f"v{side}"
                    )
                # ---- projection emitters (interleaved into early groups) ----
                def qproj(side, tch):
                    pq = pp.tile([128, 1536], F32, tag="simps", bufs=2, name=f"pq{side}{tch}")
                    for h in (0, 1):
                        for c in range(2):
                            nc.tensor.matmul(
                                pq[0:96, 512 * h : 512 * h + 512],
                                wq_sb[:, 192 * c + 96 * h : 192 * c + 96 * (h + 1)],
                                xsb[(side, c)][:, 512 * tch : 512 * (tch + 1)],
                                start=(c == 0),
                                stop=(c == 1),
                            )
                    sl = slice(512 * tch, 512 * (tch + 1))
                    for h in (0, 1):
                        nc.scalar.activation(
                            qrep[(side, h)][0:96, sl],
                            pq[0:96, 512 * h : 512 * h + 512],
                            AF.Identity,
                            bias=bq_sb[0:96, h : h + 1],
                        )

                def vproj(side, tch):
                    pv = pp.tile([128, 1536], F32, tag="simps", bufs=2, name=f"pv{side}{tch}")
                    for c in range(2):
                        nc.tensor.matmul(
                            pv[:, 0 : 2 * DA],
                            xsb[(side, c)][:, 128 * tch : 128 * (tch + 1)],
                            wv_sb[:, 2 * DA * c : 2 * DA * (c + 1)],
                            start=(c == 0),
                            stop=(c == 1),
                        )
                    nc.vector.tensor_tensor(
                        vaug[side][:, tch, :], pv[:, 0 : 2 * DA], bvt_sb[:], ALU.add
                    )

                QPB = FB // 512  # side-b q blocks consumed per fb
                def prework(gi):
                    NQ = P // 512
                    if gi < NQ:  # side-a q (lhsT) keeps pace with ls
                        qproj("a", gi)
                    # side-b q (rhs): blocks t in [QPB*f, QPB*(f+1)) needed by
                    # group NLS2*f; emit just ahead of need.
                    for t in range(NQ):
                        f = t // QPB
                        due = 0 if f == 0 else NLS2 * f - QPB + (t % QPB)
                        if gi == due:
                            qproj("b", t)
                    if gi < NT // 3:  # v projections side a, 3 tiles per group
                        for t in range(3):
                            vproj("a", 3 * gi + t)
                    elif NQ + 2 <= gi < NQ + 2 + NT // 3:
                        for t in range(3):
                            vproj("b", 3 * (gi - (NQ + 2)) + t)

                # ---- main: two orientations, software-pipelined ----
                TPG = 3  # row-tiles per sim group (3 PSUM banks, x2 buffered)
                NLS2 = P // (128 * TPG)

                # group list: (orient, fb, ls); accums lag one group behind
                # sim/exp/mask so the PE never stalls on the DVE mask.
                groups = [
                    (orient, fb, ls)
                    for orient in range(2)
                    for fb in range(NFB)
                    for ls in range(NLS2)
                ]
                maccs = {}
                mnss = {}
                live_es = {}

                def stage_front(orient, fb, ls, gi=0):
                    """mask DMA + sim + exp + mask-mult for one group."""
                    sideL = "a" if orient == 0 else "b"
                    sideR = "b" if orient == 0 else "a"
                    mdram = ma if orient == 0 else mb
                    if (orient, fb) not in maccs:
                        maccs[(orient, fb)] = pp.tile(
                            [128, FB], F32, tag="macc", bufs=MACC_BUFS, name=f"macc_{orient}_{fb}"
                        )
                    mk = wp.tile([128, TPG * FB], BF16, tag="mask", bufs=LAG + 3, name=f"mk_{orient}_{fb}_{ls}")
                    if "mdma" not in skip:
                        (nc.gpsimd if MASK_DMA_GPSIMD else nc.sync).dma_start(
                            mk.rearrange("p (t s) -> p t s", t=TPG),
                            mdram[fb, ls],
                        )
                    for pair in (0, 1):
                        es = wp.tile([128, TPG * FB], BF16, tag=f"esb{pair}", bufs=LAG + 2, name=f"es_{orient}_{fb}_{ls}_{pair}")
                        esv = es.rearrange("p (t s) -> p t s", t=TPG)
                        live_es[(orient, fb, ls, pair)] = esv
                        mkv = mk.rearrange("p (t s) -> p t s", t=TPG)
                        for c2 in ([] if "sim" in skip else range(NSC)):
                            sim = pp.tile([128, TPG * 512], F32, tag="simps", bufs=2, name=f"sim_{orient}_{fb}_{ls}_{pair}_{c2}")
                            for t in range(TPG):
                                lt = ls * TPG + t
                                nc.tensor.matmul(
                                    sim[:, 512 * t : 512 * (t + 1)],
                                    qrep[(sideL, pair)][
                                        32 * t : 32 * (t + 1),
                                        128 * lt : 128 * (lt + 1),
                                    ],
                                    qrep[(sideR, pair)][
                                        32 * t : 32 * (t + 1),
                                        FB * fb + 512 * c2 : FB * fb + 512 * (c2 + 1),
                                    ],
                                    start=True,
                                    stop=orient not in _mm_orients(),
                                    tile_position=(32 * t, 0),
                                    skip_group_check=orient in _mm_orients(),
                                )
                                if orient in _mm_orients() and "mask" not in skip:
                                    nc.tensor.matmul(
                                        sim[:, 512 * t : 512 * (t + 1)],
                                        id_sb[:, :],
                                        mkv[:, t, 512 * c2 : 512 * (c2 + 1)],
                                        start=False,
                                        stop=True,
                                        skip_group_check=True,
                                    )
                            if "exp" not in skip:
                                nc.scalar.activation(
                                    esv[:, :, 512 * c2 : 512 * (c2 + 1)],
                                    sim.rearrange("p (t s) -> p t s", t=TPG),
                                    AF.Exp,
                                    scale=1.0 / SCALE,
                                )
                        if "mask" not in skip and orient not in _mm_orients():
                            eng = nc.gpsimd if (MASK_GPS and gi % 3 == 2) else nc.vector
                            eng.tensor_tensor(es[:, :], es[:, :], mk[:, :], ALU.mult)

                def stage_accum(orient, fb, ls):
                    sideL = "a" if orient == 0 else "b"
                    vsrc = vaug[sideL]
                    macc = maccs[(orient, fb)]
                    for pair in ([] if "accum" in skip else (0, 1)):
                        esv = live_es.pop((orient, fb, ls, pair))
                        for c2 in range(NSC):
                            for t in range(TPG):
                                lt = ls * TPG + t
                                nc.tensor.matmul(
                                    macc[64 * pair : 64 * pair + DA, 512 * c2 : 512 * (c2 + 1)],
                                    vsrc[:, lt, DA * pair : DA * (pair + 1)],
                                    esv[:, t, 512 * c2 : 512 * (c2 + 1)],
                                    start=(ls == 0 and t == 0),
                                    stop=(ls == NLS2 - 1 and t == TPG - 1),
                                    tile_position=(0, 64 * pair),
                                    skip_group_check=True,
                                )

                def stage_norm(orient, fb):
                    """normalize one fb block (reads+releases the psum macc)."""
                    macc = maccs.pop((orient, fb))
                    mns = {}
                    for pair in ([] if "norm" in skip else (0, 1)):
                        rs = wp.tile([1, FB], F32, tag="rs", bufs=2, name=f"rs{pair}_{orient}_{fb}")
                        rr = wp.tile([32, FB], F32, tag="rr", bufs=2, name=f"rr{pair}_{orient}_{fb}")
                        mnh = wp.tile([32, FB], BF16, tag=f"mnh{pair}", bufs=2, name=f"mnh{pair}_{orient}_{fb}")
                        nc.vector.tensor_scalar_add(
                            rs[:], macc[64 * pair + D : 64 * pair + D + 1, :], 1e-30
                        )
                        nc.vector.reciprocal_approx_fast(rs[:], rs[:])
                        nc.gpsimd.partition_broadcast(rr[:], rs[:], channels=32)
                        nc.vector.tensor_tensor(
                            mnh[:], macc[64 * pair : 64 * pair + D, :], rr[:], ALU.mult
                        )
                        mns[pair] = mnh
                    mnss[(orient, fb)] = mns

                def stage_merge(orient, fb):
                    """merge + store one fb block (deferred; reads SBUF mns)."""
                    odram = o1 if orient == 0 else o0
                    mns = mnss.pop((orient, fb))
                    for c2 in range(NSC if "norm" not in skip else 0):
                        outsb = wp.tile([128, 1024], F32, tag="outsb", bufs=2, name=f"ou_{orient}_{fb}_{c2}")
                        for half in range(2):
                            if FB == 512:
                                merged = pp.tile([128, 512], F32, tag="mgps", bufs=1, name=f"mg_{orient}_{fb}_{c2}_{half}")
                                mg = merged[:, :]
                            else:
                                if half == 0:
                                    merged = pp.tile([128, 1536], F32, tag="simps", bufs=2, name=f"mg_{orient}_{fb}_{c2}")
                                mg = merged[:, 512 * half : 512 * (half + 1)]
                            for i2, pair in enumerate((0, 1)):
                                nc.tensor.matmul(
                                    mg,
                                    wm_sb[pair][:, 128 * half : 128 * (half + 1)],
                                    mns[pair][:, 512 * c2 : 512 * (c2 + 1)],
                                    start=(i2 == 0),
                                    stop=(i2 == 1),
                                )
                            nc.scalar.copy(
                                outsb[:, 512 * half : 512 * (half + 1)],
                                mg,
                            )
                            nc.scalar.dma_start(
                                odram[
                                    128 * half : 128 * (half + 1),
                                    FB * fb + 512 * c2 : FB * fb + 512 * (c2 + 1),
                                ],
                                outsb[:, 512 * half : 512 * (half + 1)],
                            )

                # pipelined emission: front(g) ... accum(g-1) ... norm when a
                # block's last accum has been emitted.
                pending = []
                merge_q = []  # (due_gi, orient, fb)
                for gi, g in enumerate(groups):
                    prework(gi)
                    while merge_q and merge_q[0][0] <= gi:
                        _, mo, mf = merge_q.pop(0)
                        stage_merge(mo, mf)
                    stage_front(*g, gi=gi)
                    pending.append(g)
                    if len(pending) > LAG:
                        pg = pending.pop(0)
                        stage_accum(*pg)
                        if pg[2] == NLS2 - 1:
                            stage_norm(pg[0], pg[1])
                            merge_q.append((gi + 2, pg[0], pg[1]))
                for pg in pending:
                    stage_accum(*pg)
                    if pg[2] == NLS2 - 1:
                        stage_norm(pg[0], pg[1])
                        merge_q.append((0, pg[0], pg[1]))
                for _, mo, mf in merge_q:
                    stage_merge(mo, mf)

            if reps > 1:
                import os
                if os.environ.get("KSTATIC"):
                    for _ in range(reps):
                        body()
                else:
                    with tc.For_i(0, reps, 1):
                        body()
            else:
                body()
    nc.compile()
    return nc


def _pad2(a, P):
    out = np.zeros((P, P), a.dtype)
    out[: a.shape[0], : a.shape[1]] = a
    return out


def host_prep(x0, x1, mask, W_proj, b_proj, W_merge, P=3072, FB=1024):
    """Build the 8 per-core input maps. All heavy layout work is numpy."""
    n_groups = NCORES // N  # cores per batch index
    Ls = x0.shape[1]
    NFB, NLS = P // FB, P // 512
    in_maps = []
    shared = {}
    for n in range(N):
        xpadA = np.zeros((P, C), np.float32)
        xpadA[:Ls] = x0[n]
        xpadB = np.zeros((P, C), np.float32)
        xpadB[:Ls] = x1[n]
        mA = _pad2(mask[n].astype(np.float32), P)
        # mask tiling: M[fb, ls, p, t, s] = mask[ls*384 + t*128 + p, fb*FB + s]
        def tile_mask(m, orient):
            if orient in _mm_orients():
                m = (m - 1.0) * 200.0  # 0 where kept, -200 where masked
            t = m.reshape(P // 384, 3, 128, NFB, FB).transpose(3, 0, 2, 1, 4)
            return np.ascontiguousarray(t).astype(bf16)

        shared[n] = dict(
            xa=np.ascontiguousarray(xpadA.T).astype(bf16),
            xb=np.ascontiguousarray(xpadB.T).astype(bf16),
            ma=tile_mask(mA, 0),
            mb=tile_mask(np.ascontiguousarray(mA.T), 1),
        )
    for core in range(NCORES):
        n = core // n_groups
        k = core % n_groups
        h0 = 2 * k
        heads = [h0, h0 + 1]
        wq_c = np.concatenate(
            [np.tile(W_proj[:, 32 * h : 32 * (h + 1)], (1, 3)) for h in heads], axis=1
        ).astype(bf16)
        bq_c = np.stack(
            [np.tile(b_proj[32 * h : 32 * (h + 1)], 3) for h in heads], axis=1
        ).astype(np.float32)
        # v weights with a zero column (-> +1 bias) appended per head
        wv_cols = []
        bv_vals = []
        for h in heads:
            wv_cols.append(W_proj[:, H + 32 * h : H + 32 * (h + 1)])
            wv_cols.append(np.zeros((C, 1), np.float32))
            bv_vals.extend(list(b_proj[H + 32 * h : H + 32 * (h + 1)]) + [1.0])
        wv_c = np.concatenate(wv_cols, axis=1).astype(bf16)
        bvt_c = np.tile(np.array(bv_vals, np.float32)[None, :], (128, 1))
        wm_c = np.concatenate(
            [W_merge[32 * h : 32 * (h + 1), :] for h in heads], axis=0
        ).astype(np.float32)
        wm_h = wm_c.astype(bf16)
        extra = {}
        if _mm_orients():
            extra["ident"] = np.eye(128, dtype=bf16)
        in_maps.append(
            dict(shared[n], wq=wq_c, bq=bq_c, wv=wv_c, bvt=bvt_c, wmh=wm_h, **extra)
        )
    return in_maps


_NC_CACHE = {}


def kernel(x0, x1, mask, W_proj, b_proj, W_merge, b_merge):
    x0 = np.asarray(x0, np.float32)
    x1 = np.asarray(x1, np.float32)
    mask = np.asarray(mask)
    W_proj = np.asarray(W_proj, np.float32)
    b_proj = np.asarray(b_proj, np.float32)
    W_merge = np.asarray(W_merge, np.float32)
    b_merge = np.asarray(b_merge, np.float32)

    P = 3072
    if "nc" not in _NC_CACHE:
        _NC_CACHE["nc"] = build_core(P=P)
    nc = _NC_CACHE["nc"]
    in_maps = host_prep(x0, x1, mask, W_proj, b_proj, W_merge, P=P)
    res = run_bass_kernel_spmd(nc, in_maps, core_ids=list(range(NCORES)))

    n_groups = NCORES // N
    m0 = np.zeros((N, L, C), np.float32)
    m1 = np.zeros((N, L, C), np.float32)
    for core in range(NCORES):
        n = core // n_groups
        m0[n] += res.results[core]["o0"][:, :L].T
        m1[n] += res.results[core]["o1"][:, :L].T
    m0 += b_merge
    m1 += b_merge
    return m0, m1

